# revision 39
# baseline (speedup 1.0000x reference)
"""Trainium2 Bass kernel for nn_BiaffineChart.

Computes, for x_l, x_r [1, 4096, 1024], mask [4096, 4096] (bool),
matrix [1024, 1024], wl/wr [1, 1024], bias/bl/br [1]:

    xm     = x_l @ matrix                       # [1, n, d]
    x      = xm @ x_r^T + bias                  # [1, n, n]
    x     += lin_l(x_l) + lin_r(x_r)^T          # row + col vectors
    x      = relu(x)[0]                         # [n, n]
    scores = where(mask, x, 0)
    return (scores, x)

Sharding: rows of x_l / mask / outputs split across 8 NeuronCores
(sequence parallel); matrix / wl / wr / x_r replicated.

All heavy tensors move in bf16 (half the HBM traffic of the f32r
variant) and the host pre-transposes x_l / x_r so the kernel runs zero
PE transposes:

  mm1:  xmT[r, m] = sum_l matrix[l, r] * xlT[l, m]   (lhsT = matrix in
        natural layout, rhs = host-transposed xlT).  The PSUM eviction
        adds wr[r] per partition, so mm2 picks up the lin_r column term
        for free:  (xm[m,:] + wr) . x_r[n,:] = xm.x_r + lin_r[n]
  mm2:  out[m, n] = sum_r xmT'[r, m] * xrT[r, n]     (rhs = host-
        transposed x_r, staged whole in SBUF).
  lin_l[m] + (bias+bl+br) rides in as the per-partition bias of the
  ScalarE relu that evicts mm2's PSUM.  VectorE applies the u8 mask;
  both outputs stream back to HBM as bf16 and are widened on the host.

DMA trigger discipline (the real TRN2 limiter here): every dma_start
costs ~0.6us+ of issue time (byte-bound, ~2.6us/MB), so triggers are
few, <=1MB, all inputs on SP in strict consumption order with the
tightest-margin consumers (first x_r^T blocks, which gate mm2's start)
ahead of anything that never stalls the PE (lin_l inputs, mask).
Outputs are batched 4 row-tiles per trigger on SP; the final block
streams per-tile, x on the scalar engine's queue, to shorten the drain.
"""

import os
import sys

import numpy as np

for _p in ("/opt/trn_rl_repo", "/opt/pypackages"):
    if _p not in sys.path:
        sys.path.append(_p)

import ml_dtypes
from contextlib import ExitStack

import concourse.bass as bass
import concourse.tile as tile
from concourse import bacc
from concourse import mybir
from concourse.bass_utils import run_bass_kernel_spmd

N = 4096          # sequence length (rows and cols of the chart)
D = 1024          # feature dim
NCORES = 8
MSH = N // NCORES # rows per core = 512
P = 128           # partitions
KT = D // P       # 8 k-tiles of 128
MT = MSH // P     # 4 m-tiles per core
NBLK = 8          # column blocks
NF = N // NBLK    # 512 columns per block
NWARM = 5         # PE clock-ramp warmup matmuls

F32 = mybir.dt.float32
BF16 = mybir.dt.bfloat16
U8 = mybir.dt.uint8
BF16_NP = ml_dtypes.bfloat16


def build_bass():
    nc = bacc.Bacc(name="biaffine_chart")

    xlT_d = nc.dram_tensor("xlT", [D, MSH], BF16, kind="ExternalInput")
    xl_d = nc.dram_tensor("xl", [MSH, D], BF16, kind="ExternalInput")
    xrT_d = nc.dram_tensor("xrT", [D, N], BF16, kind="ExternalInput")
    mk_d = nc.dram_tensor("mk", [MSH, N], U8, kind="ExternalInput")
    mat_d = nc.dram_tensor("mat", [D, D], BF16, kind="ExternalInput")
    wl_d = nc.dram_tensor("wl", [P, D], BF16, kind="ExternalInput")
    wrT_d = nc.dram_tensor("wrT", [P, KT], F32, kind="ExternalInput")
    c0_d = nc.dram_tensor("c0", [P, 1], F32, kind="ExternalInput")

    sc_d = nc.dram_tensor("scores", [MSH, N], BF16, kind="ExternalOutput")
    x_d = nc.dram_tensor("xout", [MSH, N], BF16, kind="ExternalOutput")

    # partitioned views: row index = tile*128 + partition
    xlT_v = xlT_d.rearrange("(ko p) m -> p ko m", p=P)   # [128, 8, 512]
    xl_v = xl_d.rearrange("(mo p) l -> p mo l", p=P)     # [128, 4, 1024]
    xrT_v = xrT_d.rearrange("(ko p) n -> p ko n", p=P)   # [128, 8, 4096]
    mk_v = mk_d.rearrange("(mo p) n -> p mo n", p=P)     # [128, 4, 4096]
    mat_v = mat_d.rearrange("(ko p) r -> p ko r", p=P)   # [128, 8, 1024]
    sc_v = sc_d.rearrange("(mo p) n -> p mo n", p=P)
    x_v = x_d.rearrange("(mo p) n -> p mo n", p=P)

    with tile.TileContext(nc) as tc, ExitStack() as ctx:
        consts = ctx.enter_context(tc.tile_pool(name="consts", bufs=1))
        big = ctx.enter_context(tc.tile_pool(name="big", bufs=1))
        out_pool = ctx.enter_context(tc.tile_pool(name="outp", bufs=4))
        mm_ps = ctx.enter_context(tc.tile_pool(name="mmps", bufs=8, space="PSUM"))

        # ---- input DMAs on SP, in consumption order.  The first mm1
        # chunks are small so the PE can start as soon as possible while
        # the DMA queues are still ramping up.
        mat_sb = big.tile([P, KT, D], BF16)
        xlT_sb = big.tile([P, KT, MSH], BF16)
        nc.sync.dma_start(mat_sb[:, 0, :], mat_v[:, 0, :])
        nc.sync.dma_start(xlT_sb[:, 0, :], xlT_v[:, 0, :])
        nc.sync.dma_start(mat_sb[:, 1, :], mat_v[:, 1, :])
        nc.sync.dma_start(xlT_sb[:, 1:4, :], xlT_v[:, 1:4, :])
        for lt in range(2, 4):
            nc.sync.dma_start(mat_sb[:, lt, :], mat_v[:, lt, :])
        nc.sync.dma_start(xlT_sb[:, 4:8, :], xlT_v[:, 4:8, :])
        for lt in range(4, KT):
            nc.sync.dma_start(mat_sb[:, lt, :], mat_v[:, lt, :])

        # small tensors go after the last mat chunks: mat lt6/lt7 arrive
        # right at mm1's consumption point, while these aren't needed
        # until the first evictions (~19us) and lin_l (~24us)
        wrT = consts.tile([P, KT], F32)
        nc.sync.dma_start(wrT[:], wrT_d[:])
        c0_sb = consts.tile([P, 1], F32)
        nc.sync.dma_start(c0_sb[:], c0_d[:])
        wl_sb = consts.tile([P, D], BF16)
        nc.sync.dma_start(wl_sb[:], wl_d[:])

        # x_r^T blocks 0-1 gate mm2's start with the thinnest margin, so
        # they go before x_l (whose lin_l consumer is buffered by 8 PSUM
        # banks of slack) and the mask (which only feeds the scores path)
        xrT_sb = big.tile([P, KT, N], BF16)
        mk_sb = big.tile([P, MT, N], U8)
        xl_sb = big.tile([P, MT, D], BF16)
        for nb in range(2):
            nc.sync.dma_start(
                xrT_sb[:, :, nb * NF:(nb + 1) * NF],
                xrT_v[:, :, nb * NF:(nb + 1) * NF],
            )
        nc.sync.dma_start(xl_sb[:], xl_v[:])
        nc.sync.dma_start(mk_sb[:, 0:2, :], mk_v[:, 0:2, :])
        nc.sync.dma_start(
            xrT_sb[:, :, 2 * NF:3 * NF], xrT_v[:, :, 2 * NF:3 * NF])
        nc.sync.dma_start(mk_sb[:, 2:4, :], mk_v[:, 2:4, :])
        for nb in range(3, NBLK):
            nc.sync.dma_start(
                xrT_sb[:, :, nb * NF:(nb + 1) * NF],
                xrT_v[:, :, nb * NF:(nb + 1) * NF],
            )

        # PE warm-up: the clock gate starts throttled and needs sustained
        # matmul activity to release; junk bf16 matmuls (no DMA dependency)
        # burn the initial DMA wait so real work starts at speed.
        warm_w = consts.tile([P, P], BF16)
        nc.vector.memset(warm_w[:], 1.0)
        warm_x = consts.tile([P, NF], BF16)
        nc.vector.memset(warm_x[:], 1.0)
        warm_ps = mm_ps.tile([P, NF], F32, tag="mm")
        for _ in range(NWARM):
            nc.tensor.matmul(warm_ps[:], warm_w[:], warm_x[:], start=True, stop=True)

        # ---- mm1: xmT[r, m] = sum_l mat[l, r] * xlT[l, m] (+ wr[r]) ----
        # lt is the outer loop (4 PSUM banks accumulate a half of the rt
        # range) so the first matmuls only need the first mat/xlT chunks
        # off the wire instead of all of them.
        xmT_sb = big.tile([P, KT, MSH], BF16)
        for half in range(2):
            ps4 = [
                mm_ps.tile([P, NF], F32, tag="mm", name=f"ps1_{half}_{j}")
                for j in range(4)
            ]
            for lt in range(KT):
                for j in range(4):
                    rt = half * 4 + j
                    nc.tensor.matmul(
                        ps4[j][:],
                        mat_sb[:, lt, rt * P:(rt + 1) * P],
                        xlT_sb[:, lt, :],
                        start=(lt == 0),
                        stop=(lt == KT - 1),
                    )
            for j in range(4):
                rt = half * 4 + j
                # second half's evictions gate mm2's first kt loop: split
                # them across DVE and ScalarE so all four banks drain in
                # half the serial time
                if half == 1 and j % 2 == 1:
                    nc.scalar.activation(
                        xmT_sb[:, rt, :], ps4[j][:],
                        mybir.ActivationFunctionType.Identity,
                        bias=wrT[:, rt:rt + 1],
                    )
                else:
                    nc.vector.tensor_scalar_add(
                        xmT_sb[:, rt, :], ps4[j][:], wrT[:, rt:rt + 1])

        # lin_l + (bias+bl+br) as a per-partition relu bias (on DVE;
        # tensor_tensor_reduce would fuse this but crashes the exec unit)
        bias_col = consts.tile([P, MT], F32)
        prod = consts.tile([P, D], F32)
        linl = consts.tile([P, MT], F32)
        for mt in range(MT):
            nc.vector.tensor_tensor(
                prod[:], xl_sb[:, mt, :], wl_sb[:], mybir.AluOpType.mult,
            )
            nc.vector.tensor_reduce(
                linl[:, mt:mt + 1], prod[:],
                mybir.AxisListType.X, mybir.AluOpType.add,
            )
            nc.vector.tensor_scalar_add(
                bias_col[:, mt:mt + 1], linl[:, mt:mt + 1], c0_sb[:, 0:1]
            )

        # ---- mm2: out[m, n] = sum_r xmT'[r, m] * xrT[r, n] ----
        # Outputs are batched 4 row-tiles per trigger; x goes out on the
        # scalar engine's HWDGE queue, masked scores on SP (idle by now).
        for nb in range(NBLK):
            x_batch = out_pool.tile([P, MT, NF], BF16, tag="xo")
            s_batch = out_pool.tile([P, MT, NF], BF16, tag="so")
            last = nb == NBLK - 1
            for mt in range(MT):
                # the very last tile's eviction chain is fully exposed in
                # the drain tail: split it 2x256 so each half's
                # relu/mask/DMA is half as long
                if last and mt == MT - 1:
                    cols = [(0, NF // 2), (NF // 2, NF)]
                else:
                    cols = [(0, NF)]
                for (ca, cb) in cols:
                    cw = cb - ca
                    ps = mm_ps.tile([P, NF], F32, tag="mm")
                    for kt in range(KT):
                        nc.tensor.matmul(
                            ps[:, 0:cw],
                            xmT_sb[:, kt, mt * P:(mt + 1) * P],
                            xrT_sb[:, kt, nb * NF + ca:nb * NF + cb],
                            start=(kt == 0),
                            stop=(kt == KT - 1),
                        )
                    nc.scalar.activation(
                        x_batch[:, mt, ca:cb], ps[:, 0:cw],
                        mybir.ActivationFunctionType.Relu,
                        bias=bias_col[:, mt:mt + 1],
                    )
                    nc.vector.tensor_tensor(
                        s_batch[:, mt, ca:cb], x_batch[:, mt, ca:cb],
                        mk_sb[:, mt, nb * NF + ca:nb * NF + cb],
                        mybir.AluOpType.mult,
                    )
                    if last:
                        # last block: per-tile DMAs start transfers early
                        nc.scalar.dma_start(
                            x_v[:, mt, nb * NF + ca:nb * NF + cb],
                            x_batch[:, mt, ca:cb])
                        nc.sync.dma_start(
                            sc_v[:, mt, nb * NF + ca:nb * NF + cb],
                            s_batch[:, mt, ca:cb])
            if not last:
                # both batch triggers go on SP: a trigger on the scalar
                # queue would delay the next block's ACTIVATE, which gates
                # PSUM recycling for the PE
                nc.sync.dma_start(x_v[:, :, nb * NF:(nb + 1) * NF], x_batch[:])
                nc.sync.dma_start(sc_v[:, :, nb * NF:(nb + 1) * NF], s_batch[:])

    nc.compile()
    return nc


_NC_CACHE = None

# test-harness knobs (the grading harness just calls kernel())
TRACE = False
TRACE_KW = {}
LAST_RESULTS = None


def _get_nc():
    global _NC_CACHE
    if _NC_CACHE is None:
        _NC_CACHE = build_bass()
    return _NC_CACHE


def kernel(x_l, x_r, mask, matrix, bias, wl, bl, wr, br, s_ind=0, **_):
    x_l = np.asarray(x_l, dtype=np.float32).reshape(N, D)
    x_r = np.asarray(x_r, dtype=np.float32).reshape(N, D)
    xl_bf = np.ascontiguousarray(x_l.astype(BF16_NP))
    xlT_bf = np.ascontiguousarray(xl_bf.T)                 # [D, N]
    xrT_bf = np.ascontiguousarray(x_r.astype(BF16_NP).T)   # [D, N]
    mat_bf = np.ascontiguousarray(
        np.asarray(matrix, dtype=np.float32).astype(BF16_NP))
    mask_u8 = np.ascontiguousarray(np.asarray(mask)).astype(np.uint8)
    wl_b = np.ascontiguousarray(np.broadcast_to(
        np.asarray(wl, dtype=np.float32).astype(BF16_NP).reshape(1, D), (P, D)))
    wrT = np.ascontiguousarray(
        np.asarray(wr, dtype=np.float32).reshape(KT, P).T)  # [P, KT]
    c0 = float(np.asarray(bias).ravel()[0]) \
        + float(np.asarray(bl).ravel()[0]) \
        + float(np.asarray(br).ravel()[0])
    c0_col = np.full((P, 1), c0, dtype=np.float32)

    nc = _get_nc()
    in_maps = []
    for c in range(NCORES):
        sl = slice(c * MSH, (c + 1) * MSH)
        in_maps.append({
            "xlT": np.ascontiguousarray(xlT_bf[:, sl]),
            "xl": xl_bf[sl],
            "xrT": xrT_bf,
            "mk": mask_u8[sl],
            "mat": mat_bf,
            "wl": wl_b,
            "wrT": wrT,
            "c0": c0_col,
        })

    res = run_bass_kernel_spmd(
        nc, in_maps, core_ids=list(range(NCORES)), trace=TRACE, **TRACE_KW
    )
    global LAST_RESULTS
    LAST_RESULTS = res
    scores = np.concatenate(
        [r["scores"] for r in res.results], axis=0).astype(np.float32)
    x = np.concatenate(
        [r["xout"] for r in res.results], axis=0).astype(np.float32)
    return (scores, x)


# revision 40
# speedup vs baseline: 1.0230x; 1.0230x over previous
"""Trainium2 Bass kernel for nn_BiaffineChart.

Computes, for x_l, x_r [1, 4096, 1024], mask [4096, 4096] (bool),
matrix [1024, 1024], wl/wr [1, 1024], bias/bl/br [1]:

    xm     = x_l @ matrix                       # [1, n, d]
    x      = xm @ x_r^T + bias                  # [1, n, n]
    x     += lin_l(x_l) + lin_r(x_r)^T          # row + col vectors
    x      = relu(x)[0]                         # [n, n]
    scores = where(mask, x, 0)
    return (scores, x)

Sharding: rows of x_l / mask / outputs split across 8 NeuronCores
(sequence parallel); matrix / wl / wr / x_r replicated.

All heavy tensors move in bf16 (half the HBM traffic of the f32r
variant) and the host pre-transposes x_l / x_r so the kernel runs zero
PE transposes:

  mm1:  xmT[r, m] = sum_l matrix[l, r] * xlT[l, m]   (lhsT = matrix in
        natural layout, rhs = host-transposed xlT).  The PSUM eviction
        adds wr[r] per partition, so mm2 picks up the lin_r column term
        for free:  (xm[m,:] + wr) . x_r[n,:] = xm.x_r + lin_r[n]
  mm2:  out[m, n] = sum_r xmT'[r, m] * xrT[r, n]     (rhs = host-
        transposed x_r, staged whole in SBUF).
  lin_l[m] + (bias+bl+br) rides in as the per-partition bias of the
  ScalarE relu that evicts mm2's PSUM.  VectorE applies the u8 mask;
  both outputs stream back to HBM as bf16 and are widened on the host.

DMA trigger discipline (the real TRN2 limiter here): every dma_start
costs ~0.6us+ of issue time (byte-bound, ~2.6us/MB), so triggers are
few, <=1MB, all inputs on SP in strict consumption order with the
tightest-margin consumers (first x_r^T blocks, which gate mm2's start)
ahead of anything that never stalls the PE (lin_l inputs, mask).
Outputs are batched 4 row-tiles per trigger on SP; the final block
streams per-tile, x on the scalar engine's queue, to shorten the drain.
"""

import os
import sys

import numpy as np

for _p in ("/opt/trn_rl_repo", "/opt/pypackages"):
    if _p not in sys.path:
        sys.path.append(_p)

import ml_dtypes
from contextlib import ExitStack

import concourse.bass as bass
import concourse.tile as tile
from concourse import bacc
from concourse import mybir
from concourse.bass_utils import run_bass_kernel_spmd

N = 4096          # sequence length (rows and cols of the chart)
D = 1024          # feature dim
NCORES = 8
MSH = N // NCORES # rows per core = 512
P = 128           # partitions
KT = D // P       # 8 k-tiles of 128
MT = MSH // P     # 4 m-tiles per core
NBLK = 8          # column blocks
NF = N // NBLK    # 512 columns per block
NWARM = 5         # PE clock-ramp warmup matmuls

F32 = mybir.dt.float32
BF16 = mybir.dt.bfloat16
U8 = mybir.dt.uint8
BF16_NP = ml_dtypes.bfloat16


def build_bass():
    nc = bacc.Bacc(name="biaffine_chart")

    xlT_d = nc.dram_tensor("xlT", [D, MSH], BF16, kind="ExternalInput")
    xl_d = nc.dram_tensor("xl", [MSH, D], BF16, kind="ExternalInput")
    xrT_d = nc.dram_tensor("xrT", [D, N], BF16, kind="ExternalInput")
    mk_d = nc.dram_tensor("mk", [MSH, N], U8, kind="ExternalInput")
    mat_d = nc.dram_tensor("mat", [D, D], BF16, kind="ExternalInput")
    wl_d = nc.dram_tensor("wl", [P, D], BF16, kind="ExternalInput")
    wrT_d = nc.dram_tensor("wrT", [P, KT], F32, kind="ExternalInput")
    c0_d = nc.dram_tensor("c0", [P, 1], F32, kind="ExternalInput")

    sc_d = nc.dram_tensor("scores", [MSH, N], BF16, kind="ExternalOutput")
    x_d = nc.dram_tensor("xout", [MSH, N], BF16, kind="ExternalOutput")

    # partitioned views: row index = tile*128 + partition
    xlT_v = xlT_d.rearrange("(ko p) m -> p ko m", p=P)   # [128, 8, 512]
    xl_v = xl_d.rearrange("(mo p) l -> p mo l", p=P)     # [128, 4, 1024]
    xrT_v = xrT_d.rearrange("(ko p) n -> p ko n", p=P)   # [128, 8, 4096]
    mk_v = mk_d.rearrange("(mo p) n -> p mo n", p=P)     # [128, 4, 4096]
    mat_v = mat_d.rearrange("(ko p) r -> p ko r", p=P)   # [128, 8, 1024]
    sc_v = sc_d.rearrange("(mo p) n -> p mo n", p=P)
    x_v = x_d.rearrange("(mo p) n -> p mo n", p=P)

    with tile.TileContext(nc) as tc, ExitStack() as ctx:
        consts = ctx.enter_context(tc.tile_pool(name="consts", bufs=1))
        big = ctx.enter_context(tc.tile_pool(name="big", bufs=1))
        out_pool = ctx.enter_context(tc.tile_pool(name="outp", bufs=4))
        mm_ps = ctx.enter_context(tc.tile_pool(name="mmps", bufs=8, space="PSUM"))

        # ---- input DMAs on SP, in consumption order.  The first mm1
        # chunks are small so the PE can start as soon as possible while
        # the DMA queues are still ramping up.
        mat_sb = big.tile([P, KT, D], BF16)
        xlT_sb = big.tile([P, KT, MSH], BF16)
        nc.sync.dma_start(mat_sb[:, 0, :], mat_v[:, 0, :])
        nc.sync.dma_start(xlT_sb[:, 0, :], xlT_v[:, 0, :])
        nc.sync.dma_start(mat_sb[:, 1, :], mat_v[:, 1, :])
        nc.sync.dma_start(xlT_sb[:, 1:4, :], xlT_v[:, 1:4, :])
        for lt in range(2, 4):
            nc.sync.dma_start(mat_sb[:, lt, :], mat_v[:, lt, :])
        nc.sync.dma_start(xlT_sb[:, 4:8, :], xlT_v[:, 4:8, :])
        for lt in range(4, KT):
            nc.sync.dma_start(mat_sb[:, lt, :], mat_v[:, lt, :])

        # small tensors go after the last mat chunks: mat lt6/lt7 arrive
        # right at mm1's consumption point, while these aren't needed
        # until the first evictions (~19us) and lin_l (~24us)
        wrT = consts.tile([P, KT], F32)
        nc.sync.dma_start(wrT[:], wrT_d[:])
        c0_sb = consts.tile([P, 1], F32)
        nc.sync.dma_start(c0_sb[:], c0_d[:])
        wl_sb = consts.tile([P, D], BF16)
        nc.sync.dma_start(wl_sb[:], wl_d[:])

        # x_r^T blocks 0-1 gate mm2's start with the thinnest margin, so
        # they go before x_l (whose lin_l consumer is buffered by 8 PSUM
        # banks of slack) and the mask (which only feeds the scores path)
        xrT_sb = big.tile([P, KT, N], BF16)
        mk_sb = big.tile([P, MT, N], U8)
        xl_sb = big.tile([P, MT, D], BF16)
        for nb in range(2):
            nc.sync.dma_start(
                xrT_sb[:, :, nb * NF:(nb + 1) * NF],
                xrT_v[:, :, nb * NF:(nb + 1) * NF],
            )
        nc.sync.dma_start(xl_sb[:], xl_v[:])
        nc.sync.dma_start(mk_sb[:, 0:2, :], mk_v[:, 0:2, :])
        nc.sync.dma_start(
            xrT_sb[:, :, 2 * NF:3 * NF], xrT_v[:, :, 2 * NF:3 * NF])
        nc.sync.dma_start(mk_sb[:, 2:4, :], mk_v[:, 2:4, :])
        for nb in range(3, NBLK):
            nc.sync.dma_start(
                xrT_sb[:, :, nb * NF:(nb + 1) * NF],
                xrT_v[:, :, nb * NF:(nb + 1) * NF],
            )

        # PE warm-up: the clock gate starts throttled and needs sustained
        # matmul activity to release; junk bf16 matmuls (no DMA dependency)
        # burn the initial DMA wait so real work starts at speed.
        warm_w = consts.tile([P, P], BF16)
        nc.vector.memset(warm_w[:], 1.0)
        warm_x = consts.tile([P, NF], BF16)
        nc.vector.memset(warm_x[:], 1.0)
        warm_ps = mm_ps.tile([P, NF], F32, tag="mm")
        for _ in range(NWARM):
            nc.tensor.matmul(warm_ps[:], warm_w[:], warm_x[:], start=True, stop=True)

        # ---- mm1: xmT[r, m] = sum_l mat[l, r] * xlT[l, m] (+ wr[r]) ----
        # lt is the outer loop (4 PSUM banks accumulate a half of the rt
        # range) so the first matmuls only need the first mat/xlT chunks
        # off the wire instead of all of them.
        xmT_sb = big.tile([P, KT, MSH], BF16)
        for half in range(2):
            ps4 = [
                mm_ps.tile([P, NF], F32, tag="mm", name=f"ps1_{half}_{j}")
                for j in range(4)
            ]
            for lt in range(KT):
                for j in range(4):
                    rt = half * 4 + j
                    nc.tensor.matmul(
                        ps4[j][:],
                        mat_sb[:, lt, rt * P:(rt + 1) * P],
                        xlT_sb[:, lt, :],
                        start=(lt == 0),
                        stop=(lt == KT - 1),
                    )
            for j in range(4):
                rt = half * 4 + j
                # second half's evictions gate mm2's first kt loop: split
                # them across DVE and ScalarE so all four banks drain in
                # half the serial time
                if half == 1 and j % 2 == 1:
                    nc.scalar.activation(
                        xmT_sb[:, rt, :], ps4[j][:],
                        mybir.ActivationFunctionType.Identity,
                        bias=wrT[:, rt:rt + 1],
                    )
                else:
                    nc.vector.tensor_scalar_add(
                        xmT_sb[:, rt, :], ps4[j][:], wrT[:, rt:rt + 1])

        # lin_l + (bias+bl+br) as a per-partition relu bias (on DVE;
        # tensor_tensor_reduce would fuse this but crashes the exec unit)
        bias_col = consts.tile([P, MT], F32)
        prod = consts.tile([P, D], F32)
        linl = consts.tile([P, MT], F32)
        for mt in range(MT):
            nc.vector.tensor_tensor(
                prod[:], xl_sb[:, mt, :], wl_sb[:], mybir.AluOpType.mult,
            )
            nc.vector.tensor_reduce(
                linl[:, mt:mt + 1], prod[:],
                mybir.AxisListType.X, mybir.AluOpType.add,
            )
            nc.vector.tensor_scalar_add(
                bias_col[:, mt:mt + 1], linl[:, mt:mt + 1], c0_sb[:, 0:1]
            )

        # ---- mm2: out[m, n] = sum_r xmT'[r, m] * xrT[r, n] ----
        # Outputs are batched 4 row-tiles per trigger; x goes out on the
        # scalar engine's HWDGE queue, masked scores on SP (idle by now).
        for nb in range(NBLK):
            x_batch = out_pool.tile([P, MT, NF], BF16, tag="xo")
            s_batch = out_pool.tile([P, MT, NF], BF16, tag="so")
            last = nb == NBLK - 1
            cols = [(0, NF)]
            for mt in range(MT):
                for (ca, cb) in cols:
                    cw = cb - ca
                    ps = mm_ps.tile([P, NF], F32, tag="mm")
                    for kt in range(KT):
                        nc.tensor.matmul(
                            ps[:, 0:cw],
                            xmT_sb[:, kt, mt * P:(mt + 1) * P],
                            xrT_sb[:, kt, nb * NF + ca:nb * NF + cb],
                            start=(kt == 0),
                            stop=(kt == KT - 1),
                        )
                    nc.scalar.activation(
                        x_batch[:, mt, ca:cb], ps[:, 0:cw],
                        mybir.ActivationFunctionType.Relu,
                        bias=bias_col[:, mt:mt + 1],
                    )
                    nc.vector.tensor_tensor(
                        s_batch[:, mt, ca:cb], x_batch[:, mt, ca:cb],
                        mk_sb[:, mt, nb * NF + ca:nb * NF + cb],
                        mybir.AluOpType.mult,
                    )
                    if last:
                        # last block: per-tile DMAs start transfers early
                        nc.scalar.dma_start(
                            x_v[:, mt, nb * NF + ca:nb * NF + cb],
                            x_batch[:, mt, ca:cb])
                        nc.sync.dma_start(
                            sc_v[:, mt, nb * NF + ca:nb * NF + cb],
                            s_batch[:, mt, ca:cb])
            if not last:
                # both batch triggers go on SP: a trigger on the scalar
                # queue would delay the next block's ACTIVATE, which gates
                # PSUM recycling for the PE
                nc.sync.dma_start(x_v[:, :, nb * NF:(nb + 1) * NF], x_batch[:])
                nc.sync.dma_start(sc_v[:, :, nb * NF:(nb + 1) * NF], s_batch[:])

    nc.compile()
    return nc


_NC_CACHE = None

# test-harness knobs (the grading harness just calls kernel())
TRACE = False
TRACE_KW = {}
LAST_RESULTS = None


def _get_nc():
    global _NC_CACHE
    if _NC_CACHE is None:
        _NC_CACHE = build_bass()
    return _NC_CACHE


def kernel(x_l, x_r, mask, matrix, bias, wl, bl, wr, br, s_ind=0, **_):
    x_l = np.asarray(x_l, dtype=np.float32).reshape(N, D)
    x_r = np.asarray(x_r, dtype=np.float32).reshape(N, D)
    xl_bf = np.ascontiguousarray(x_l.astype(BF16_NP))
    xlT_bf = np.ascontiguousarray(xl_bf.T)                 # [D, N]
    xrT_bf = np.ascontiguousarray(x_r.astype(BF16_NP).T)   # [D, N]
    mat_bf = np.ascontiguousarray(
        np.asarray(matrix, dtype=np.float32).astype(BF16_NP))
    mask_u8 = np.ascontiguousarray(np.asarray(mask)).astype(np.uint8)
    wl_b = np.ascontiguousarray(np.broadcast_to(
        np.asarray(wl, dtype=np.float32).astype(BF16_NP).reshape(1, D), (P, D)))
    wrT = np.ascontiguousarray(
        np.asarray(wr, dtype=np.float32).reshape(KT, P).T)  # [P, KT]
    c0 = float(np.asarray(bias).ravel()[0]) \
        + float(np.asarray(bl).ravel()[0]) \
        + float(np.asarray(br).ravel()[0])
    c0_col = np.full((P, 1), c0, dtype=np.float32)

    nc = _get_nc()
    in_maps = []
    for c in range(NCORES):
        sl = slice(c * MSH, (c + 1) * MSH)
        in_maps.append({
            "xlT": np.ascontiguousarray(xlT_bf[:, sl]),
            "xl": xl_bf[sl],
            "xrT": xrT_bf,
            "mk": mask_u8[sl],
            "mat": mat_bf,
            "wl": wl_b,
            "wrT": wrT,
            "c0": c0_col,
        })

    res = run_bass_kernel_spmd(
        nc, in_maps, core_ids=list(range(NCORES)), trace=TRACE, **TRACE_KW
    )
    global LAST_RESULTS
    LAST_RESULTS = res
    scores = np.concatenate(
        [r["scores"] for r in res.results], axis=0).astype(np.float32)
    x = np.concatenate(
        [r["xout"] for r in res.results], axis=0).astype(np.float32)
    return (scores, x)
